# revision 12
# baseline (speedup 1.0000x reference)
"""Trainium2 Bass kernel for nn_CorefModel (LSTM + span pooling + mention MLP +
windowed pairwise precedent MLP + softmax).

Sharding: data-parallel over batch B=8 across the 8 NeuronCores (one batch row
per core, all parameters replicated). No collectives.

Per-core pipeline (all layouts transposed so the partition dim is 128):
  A) indirect-DMA embedding gather -> fp16 -> DRAM -> transposing DMA -> we^T
  B) X^T = Wih^T @ we^T + bias (ones-row trick) -> PSUM -> XT in SBUF,
     step-major layout [128, t, gate-chunk].
  C) 512-step LSTM recurrence. X is DMA'd (idle sync engine) from SBUF into
     2 rotating PSUM banks in 64-step windows; the 16 per-step matmuls
     accumulate Whh @ h on top, so gates are read straight out of PSUM with
     one tanh [128,2] + one sigmoid [128,6] (bank col order g g i i f f o o).
     4 DVE ops/step; h written once, fp16, into the seq^T history.
  D/E/F interleaved into the LSTM's idle engine windows: mentions whose
     spans (and precedent windows) end before step 384 have their pooling,
     mention-MLP, pairwise features and pairwise MLP emitted as filler
     between LSTM steps (the schedule is computed from the actual inputs at
     build time — the LSTM keeps every engine <50% busy, so filler rides
     free and keeps engines warm). The remainder runs after the last step.
     Pairwise features use sliding-window APs of tgt^T directly as matmul
     rhs (only the elementwise product chunks are materialized); the h2 /
     head loops are fused per 500-pair block.
  G) scores + masked softmax; epsilon col = -ms_i via shift-invariance.
"""
import numpy as np

B, W, M, P = 8, 512, 128, 50
V, E, L, H = 50000, 300, 256, 512
G = 4 * L
NCORES = 8
NEG_INF = -1.0e30

_CACHE = {}


# ---------------------------------------------------------------- host prep --
def _perm_banks():
    """Device gate col order per step: [g0 g1 i0 i1 f0 f1 o0 o1] (chunks of
    128; halves of L=256). Reference gate order is (i, f, g, o)."""
    return np.concatenate([np.arange(512, 768), np.arange(0, 256),
                           np.arange(256, 512), np.arange(768, 1024)])


def _blocked(w, kchunks, hchunks):
    """[K,HH] -> [128, kchunks*hchunks*128] with col block (k*hchunks+h)*128."""
    K, HH = w.shape
    out = np.zeros((128, kchunks * hchunks * 128), w.dtype)
    for k in range(kchunks):
        kp = min(128, K - k * 128)
        for h in range(hchunks):
            blk = w[k * 128:k * 128 + kp, h * 128:(h + 1) * 128]
            out[:kp, (k * hchunks + h) * 128:(k * hchunks + h + 1) * 128] = blk
    return out


def _chunk_cols(v, n):
    """[n*128] -> [128, n] (col j = chunk j)."""
    return np.ascontiguousarray(v.reshape(n, 128).T)


def _prep_shared(inputs):
    f32, f16 = np.float32, np.float16
    perm = _perm_banks()
    Wih = np.asarray(inputs["Wih"], f32)[:, perm]
    Whh = np.asarray(inputs["Whh"], f32)[:, perm]
    bias = (np.asarray(inputs["bih"], f32) + np.asarray(inputs["bhh"], f32))[perm]

    # rows 0:300 = Wih, row 300 = bias (matched by a ones-row in we^T)
    wih_pad = np.zeros((304, G), f16)
    wih_pad[:E] = Wih.astype(f16)
    wih_pad[E] = bias.astype(f16)

    i_idx = np.arange(M)[:, None]
    k_idx = np.arange(P)[None, :]
    valid = k_idx < np.minimum(i_idx, P)
    maskinf = np.where(valid, 0.0, NEG_INF).astype(f32)

    return {
        "emb": np.asarray(inputs["emb"], f32),
        "wih16": wih_pad,
        "whh16": Whh.astype(f16),
        "wm1": _blocked(np.asarray(inputs["Wm1"], f32), 2, 4),
        "wm2": _blocked(np.asarray(inputs["Wm2"], f32), 4, 4),
        "bm": np.concatenate([_chunk_cols(np.asarray(inputs["bm1"], f32), 4),
                              _chunk_cols(np.asarray(inputs["bm2"], f32), 4)], 1),
        "wmv": _chunk_cols(np.asarray(inputs["wm"], f32), 4),
        "wa1": _blocked(np.asarray(inputs["Wa1"], np.float32).astype(f16), 6, 4),
        "wa2": _blocked(np.asarray(inputs["Wa2"], np.float32).astype(f16), 4, 4),
        "ba": np.concatenate([_chunk_cols(np.asarray(inputs["ba1"], f32), 4),
                              _chunk_cols(np.asarray(inputs["ba2"], f32), 4)], 1),
        "wav": _chunk_cols(np.asarray(inputs["wa"], np.float32), 4).astype(f16),
        "maskinf": maskinf,
        "ident16": np.eye(128, dtype=f16),
    }


def _prep_core(inputs, b):
    f32 = np.float32
    word = np.asarray(inputs["word_seq"][b], np.int32)
    starts = np.asarray(inputs["span_starts"][b], np.int64)
    lens = np.asarray(inputs["span_lengths"][b], np.int64)
    ends = np.clip(starts + lens, 0, W)
    t_idx = np.arange(W)[:, None]
    ind_full = ((t_idx >= starts[None, :]) & (t_idx < ends[None, :])).astype(f32)
    # ind[p, q*128+m] = ind_full[q*128+p, m]
    ind = np.ascontiguousarray(
        ind_full.reshape(4, 128, M).transpose(1, 0, 2).reshape(128, 4 * M)
    ).astype(np.float16)
    widx = np.ascontiguousarray(word.reshape(4, 128).T).astype(np.int32)
    return {"widx": widx, "ind": ind}


# ------------------------------------------------------------ program build --
def _build_program(i0):
    """i0 = number of early mentions (multiple of 10): every mention m < i0
    has max-over-cores span end <= 384, so its pooled repr + pairwise rows
    are final after LSTM step 383 and can interleave with steps 384..511."""
    import concourse.bacc as bacc
    import concourse.tile as tile
    from concourse import mybir
    import concourse.bass as bass

    f32, f16, i32 = mybir.dt.float32, mybir.dt.float16, mybir.dt.int32
    AF = mybir.ActivationFunctionType
    OP = mybir.AluOpType

    nc = bacc.Bacc("TRN2", target_bir_lowering=False, debug=False)

    def din(name, shape, dt):
        return nc.dram_tensor(name, shape, dt, kind="ExternalInput").ap()

    emb_d = din("emb", [V, E], f32)
    widx_d = din("widx", [128, 4], i32)
    wih_d = din("wih16", [304, G], f16)
    whh_d = din("whh16", [L, G], f16)
    ind_d = din("ind", [128, 4 * M], f16)
    wm1_d = din("wm1", [128, 2 * 4 * 128], f32)
    wm2_d = din("wm2", [128, 4 * 4 * 128], f32)
    bm_d = din("bm", [128, 8], f32)
    wmv_d = din("wmv", [128, 4], f32)
    wa1_d = din("wa1", [128, 6 * 4 * 128], f16)
    wa2_d = din("wa2", [128, 4 * 4 * 128], f16)
    ba_d = din("ba", [128, 8], f32)
    wav_d = din("wav", [128, 4], f16)
    mask_d = din("maskinf", [128, P], f32)
    ident_d = din("ident16", [128, 128], f16)

    we16_d = nc.dram_tensor("we16s", [W, 384], f16).ap()
    ms_d = nc.dram_tensor("mss", [M, 1], f32).ap()
    ps_d = nc.dram_tensor("pss", [1, M * P], f32).ap()
    out_d = nc.dram_tensor("o", [M, P + 1], f32, kind="ExternalOutput").ap()

    def ap3(base, off_elems, dims):
        """Manual AP on the same tensor: dims = [[stride, num], ...] (free),
        partition dim copied from base."""
        return bass.AP(tensor=base.tensor, offset=base.offset + off_elems,
                       ap=[base.ap[0]] + dims)

    # pairwise 500-pair blocks: block n covers mentions i in [10n, 10n+10)
    NPAIR = M * P
    BLKS = []
    for n in range(13):
        c0 = 500 * n
        nb = min(500, NPAIR - c0)
        BLKS.append((n, c0, nb, nb // P))
    n_early = i0 // 10

    with tile.TileContext(nc) as tc:
        from contextlib import ExitStack
        ctx = ExitStack()
        with ctx:
            singles = ctx.enter_context(tc.tile_pool(name="singles", bufs=1))

            weT = singles.tile([128, 3, W], f16)
            wih_sb = singles.tile([128, 3, 8, 128], f16)
            whh_sb = singles.tile([128, 2, 8, 128], f16)
            seqT = singles.tile([128, 2, W], f16)
            ident_sb = singles.tile([128, 128], f16)
            ind_sb = singles.tile([128, 4, M], f16)
            c32 = singles.tile([128, 2], f32)
            XT = singles.tile([128, W, 8], f32)   # step-major X (+bias)

            wm1_sb = singles.tile([128, 2, 4, 128], f32)
            wm2_sb = singles.tile([128, 4, 4, 128], f32)
            bm_sb = singles.tile([128, 8], f32)
            wmv_sb = singles.tile([128, 4], f32)
            wa1_sb = singles.tile([128, 6, 4, 128], f16)
            wa2_sb = singles.tile([128, 4, 4, 128], f16)
            ba_sb = singles.tile([128, 8], f32)
            wav_sb = singles.tile([128, 4], f16)
            mask_sb = singles.tile([128, P], f32)
            tgtT32 = singles.tile([128, 2, M], f32)
            tgtT16 = singles.tile([128, 2, M], f16)
            tgt16 = singles.tile([128, 256], f16)
            m1T = singles.tile([128, 4, M], f32)
            m2T = singles.tile([128, 4, M], f32)
            prodT = singles.tile([128, 2, NPAIR], f16)
            h1T = singles.tile([128, 4, NPAIR], f16)
            ms_sb = singles.tile([1, M], f32)
            msi_sb = singles.tile([128, 1], f32)
            msj_sb = singles.tile([128, P], f32)
            psM_sb = singles.tile([128, P], f32)
            idx_sb = singles.tile([128, 4], i32)

            # weight / static DMAs (no deps -> scheduled early)
            nc.sync.dma_start(out=idx_sb[:], in_=widx_d[:])
            for k in range(3):
                kp = 128 if k < 2 else 48
                nc.sync.dma_start(out=wih_sb[0:kp, k, :, :],
                                  in_=wih_d[k * 128:k * 128 + kp, :])
            for k in range(2):
                nc.sync.dma_start(out=whh_sb[:, k, :, :],
                                  in_=whh_d[k * 128:(k + 1) * 128, :])
            nc.sync.dma_start(out=ident_sb[:], in_=ident_d[:])
            nc.sync.dma_start(out=ind_sb[:], in_=ind_d[:])
            nc.sync.dma_start(out=wm1_sb[:], in_=wm1_d[:])
            nc.sync.dma_start(out=wm2_sb[:], in_=wm2_d[:])
            nc.sync.dma_start(out=bm_sb[:], in_=bm_d[:])
            nc.sync.dma_start(out=wmv_sb[:], in_=wmv_d[:])
            nc.sync.dma_start(out=wa1_sb[:], in_=wa1_d[:])
            nc.sync.dma_start(out=wa2_sb[:], in_=wa2_d[:])
            nc.sync.dma_start(out=ba_sb[:], in_=ba_d[:])
            nc.sync.dma_start(out=wav_sb[:], in_=wav_d[:])
            nc.sync.dma_start(out=mask_sb[:], in_=mask_d[:])

            # ---- phase A: embedding gather + transpose -----------------------
            with tc.tile_pool(name="gath", bufs=2) as gpool:
                for g in range(4):
                    wet = gpool.tile([128, 384], f32, tag="wet")
                    # col 300 = ones (matches the bias row of wih); rest pad 0
                    nc.vector.memset(wet[:, E:E + 1], 1.0)
                    nc.vector.memset(wet[:, E + 1:384], 0.0)
                    nc.gpsimd.indirect_dma_start(
                        out=wet[:, 0:E], out_offset=None, in_=emb_d[:],
                        in_offset=bass.IndirectOffsetOnAxis(
                            ap=idx_sb[:, g:g + 1], axis=0))
                    nc.gpsimd.dma_start(out=we16_d[g * 128:(g + 1) * 128, :],
                                        in_=wet[:])
                for c in range(3):
                    nc.sync.dma_start(out=weT[:, c, :],
                                      in_=we16_d[:, c * 128:(c + 1) * 128],
                                      transpose=True)

            # ---- phase B: X^T + bias -> PSUM -> XT (step-major) --------------
            with tc.tile_pool(name="bps", bufs=2, space="PSUM") as bps:
                for j in range(8):
                    bx = bps.tile([128, W], f32, tag="bx")
                    for k, kp in enumerate([128, 128, 45]):
                        nc.tensor.matmul(out=bx[:], lhsT=wih_sb[0:kp, k, j, :],
                                         rhs=weT[0:kp, k, :],
                                         start=(k == 0), stop=(k == 2))
                    nc.vector.tensor_copy(out=XT[:, :, j], in_=bx[:])

            # ---- phases C + interleaved D/E/F --------------------------------
            with tc.tile_pool(name="xwin", bufs=1, space="PSUM") as xwp, \
                 tc.tile_pool(name="tps", bufs=1, space="PSUM") as tps, \
                 tc.tile_pool(name="ptp", bufs=1, space="PSUM") as ptp, \
                 tc.tile_pool(name="fps", bufs=2, space="PSUM") as fps, \
                 tc.tile_pool(name="hps", bufs=1, space="PSUM") as hps, \
                 tc.tile_pool(name="lsb", bufs=3) as lsb, \
                 tc.tile_pool(name="dsb", bufs=2) as dsb, \
                 tc.tile_pool(name="h2p", bufs=2) as h2p, \
                 tc.tile_pool(name="fpssb", bufs=3) as fps_sb:
                xwin = xwp.tile([128, 2, 64, 8], f32)
                tgt_ps = tps.tile([128, 256], f32)

                def win_dma(w):
                    # DMA cannot target PSUM; one DVE copy per 64 steps
                    nc.vector.tensor_copy(out=xwin[:, w % 2, :, :],
                                          in_=XT[:, 64 * w:64 * w + 64, :])

                win_dma(0)
                win_dma(1)

                # ---------------- filler thunk machinery ----------------------
                pending = []

                def drain(n):
                    for _ in range(n):
                        if not pending:
                            return
                        pending.pop(0)()

                def pool_q(q, start, stop):
                    def t1():
                        pool_q.seq_q = dsb.tile([128, 2, 128], f16, tag="seqq")
                        for c in range(2):
                            pt = ptp.tile([128, 128], f16, tag="pt",
                                          name=f"pt_{q}_{c}")
                            nc.tensor.transpose(
                                out=pt[:], in_=seqT[:, c, q * 128:(q + 1) * 128],
                                identity=ident_sb[:])
                            nc.vector.tensor_copy(out=pool_q.seq_q[:, c, :],
                                                  in_=pt[:])

                    def t2():
                        nc.tensor.matmul(
                            out=tgt_ps[:], lhsT=ind_sb[:, q, :],
                            rhs=pool_q.seq_q[:].rearrange("p c t -> p (c t)"),
                            start=start, stop=stop, skip_group_check=True)
                    return [t1, t2]

                def tgt_out():
                    def t1():
                        nc.vector.tensor_copy(out=tgt16[:], in_=tgt_ps[:])

                    def mk(c):
                        def t2():
                            pt2 = ptp.tile([128, 128], f16, tag="pt",
                                           name=f"pt2_{c}_{tgt_out.n}")
                            nc.tensor.transpose(
                                out=pt2[:], in_=tgt16[:, c * 128:(c + 1) * 128],
                                identity=ident_sb[:])
                            nc.vector.tensor_copy(out=tgtT32[:, c, :], in_=pt2[:])
                            nc.vector.tensor_copy(out=tgtT16[:, c, :], in_=pt2[:])
                        return t2
                    tgt_out.n += 1
                    return [t1, mk(0), mk(1)]
                tgt_out.n = 0

                def mention_mlp(m_lo, m_hi):
                    out = []
                    nm = m_hi - m_lo

                    def mk1(h):
                        def t():
                            pm = fps.tile([128, 500], f32, tag="p1",
                                          name=f"pm1_{h}_{m_lo}")
                            for k in range(2):
                                nc.tensor.matmul(out=pm[:, 0:nm],
                                                 lhsT=wm1_sb[:, k, h, :],
                                                 rhs=tgtT32[:, k, m_lo:m_hi],
                                                 start=(k == 0), stop=(k == 1))
                            nc.scalar.activation(out=m1T[:, h, m_lo:m_hi],
                                                 in_=pm[:, 0:nm], func=AF.Relu,
                                                 bias=bm_sb[:, h:h + 1])
                        return t

                    def mk2(h):
                        def t():
                            pm = fps.tile([128, 500], f32, tag="p1",
                                          name=f"pm2_{h}_{m_lo}")
                            for k in range(4):
                                nc.tensor.matmul(out=pm[:, 0:nm],
                                                 lhsT=wm2_sb[:, k, h, :],
                                                 rhs=m1T[:, k, m_lo:m_hi],
                                                 start=(k == 0), stop=(k == 3))
                            nc.scalar.activation(out=m2T[:, h, m_lo:m_hi],
                                                 in_=pm[:, 0:nm], func=AF.Relu,
                                                 bias=bm_sb[:, 4 + h:5 + h])
                        return t
                    for h in range(4):
                        out.append(mk1(h))
                    for h in range(4):
                        out.append(mk2(h))
                    return out

                def jvec_view(c, n, c0, nb, ni):
                    base = tgtT16[:, c, :]
                    if n < 5:
                        return ap3(base, 0, [[0, ni], [1, P]])
                    return ap3(base, 10 * n - P, [[1, ni], [1, P]])

                def ivec_view(c, n, c0, nb, ni):
                    base = tgtT16[:, c, :]
                    return ap3(base, 10 * n, [[1, ni], [0, P]])

                def prod_blk(n, c0, nb, ni):
                    def mk(c):
                        def t():
                            nc.vector.tensor_tensor(
                                out=prodT[:, c, c0:c0 + nb].rearrange(
                                    "p (i k) -> p i k", k=P),
                                in0=jvec_view(c, n, c0, nb, ni),
                                in1=ivec_view(c, n, c0, nb, ni), op=OP.mult)
                        return t
                    return [mk(0), mk(1)]

                def h1_blk(n, c0, nb, ni):
                    out = []

                    def mk(h):
                        def rhs(k):
                            if k < 2:
                                return jvec_view(k, n, c0, nb, ni)
                            if k < 4:
                                return ivec_view(k - 2, n, c0, nb, ni)
                            return prodT[:, k - 4, c0:c0 + nb]

                        def t1():
                            p1 = fps.tile([128, 500], f32, tag="p1",
                                          name=f"ph1a_{n}_{h}")
                            h1_blk.cur = p1
                            for k in range(3):
                                nc.tensor.matmul(out=p1[:, 0:nb],
                                                 lhsT=wa1_sb[:, k, h, :],
                                                 rhs=rhs(k),
                                                 start=(k == 0), stop=False)

                        def t2():
                            p1 = h1_blk.cur
                            for k in range(3, 6):
                                nc.tensor.matmul(out=p1[:, 0:nb],
                                                 lhsT=wa1_sb[:, k, h, :],
                                                 rhs=rhs(k),
                                                 start=False, stop=(k == 5))
                            nc.scalar.activation(out=h1T[:, h, c0:c0 + nb],
                                                 in_=p1[:, 0:nb], func=AF.Relu,
                                                 bias=ba_sb[:, h:h + 1])
                        return [t1, t2]
                    for h in range(4):
                        out += mk(h)
                    return out

                def h2_blk(n, c0, nb, ni):
                    out = []

                    def alloc():
                        h2_blk.cur = h2p.tile([128, 4, 500], f16, tag="h2b",
                                              name=f"h2b_{n}")

                    def mk(h):
                        def t():
                            h2b = h2_blk.cur
                            p2 = fps.tile([128, 500], f32, tag="p1",
                                          name=f"ph2_{n}_{h}")
                            for k in range(4):
                                nc.tensor.matmul(out=p2[:, 0:nb],
                                                 lhsT=wa2_sb[:, k, h, :],
                                                 rhs=h1T[:, k, c0:c0 + nb],
                                                 start=(k == 0), stop=(k == 3))
                            nc.scalar.activation(out=h2b[:, h, 0:nb],
                                                 in_=p2[:, 0:nb], func=AF.Relu,
                                                 bias=ba_sb[:, 4 + h:5 + h])
                        return t

                    def head():
                        h2b = h2_blk.cur
                        pps = hps.tile([1, 500], f32, tag="pps",
                                       name=f"pps_{n}")
                        for k in range(4):
                            nc.tensor.matmul(out=pps[:, 0:nb],
                                             lhsT=wav_sb[:, k:k + 1],
                                             rhs=h2b[:, k, 0:nb],
                                             start=(k == 0), stop=(k == 3))
                        pse = fps_sb.tile([1, 500], f32, tag="pse",
                                          name=f"pse_{n}")
                        nc.vector.tensor_copy(out=pse[:, 0:nb], in_=pps[:, 0:nb])
                        nc.sync.dma_start(out=ps_d[:, c0:c0 + nb],
                                          in_=pse[:, 0:nb])
                    out.append(alloc)
                    for h in range(4):
                        out.append(mk(h))
                    out.append(head)
                    return out

                # ---------------- the LSTM loop -------------------------------
                nc.vector.memset(c32[:], 0.0)
                for t in range(W):
                    win, s = (t >> 6) & 1, t & 63
                    if t > 0:
                        for k in range(2):
                            for j in range(8):
                                nc.tensor.matmul(
                                    out=xwin[:, win, s, j:j + 1],
                                    lhsT=whh_sb[:, k, j, :],
                                    rhs=seqT[:, k, t - 1:t],
                                    start=False, stop=(k == 1),
                                    skip_group_check=True)
                    ga = lsb.tile([128, 8], f32, tag="ga")
                    nc.scalar.activation(out=ga[:, 0:2],
                                         in_=xwin[:, win, s, 0:2], func=AF.Tanh)
                    nc.scalar.activation(out=ga[:, 2:8],
                                         in_=xwin[:, win, s, 2:8],
                                         func=AF.Sigmoid)
                    tct = lsb.tile([128, 2], f32, tag="tc")
                    if t == 0:
                        nc.vector.tensor_tensor(out=c32[:], in0=ga[:, 0:2],
                                                in1=ga[:, 2:4], op=OP.mult)
                    else:
                        igfc = lsb.tile([128, 4], f32, tag="igfc")
                        nc.vector.tensor_tensor(out=igfc[:, 0:2], in0=ga[:, 0:2],
                                                in1=ga[:, 2:4], op=OP.mult)
                        nc.vector.tensor_tensor(out=igfc[:, 2:4], in0=ga[:, 4:6],
                                                in1=c32[:], op=OP.mult)
                        nc.vector.tensor_tensor(out=c32[:], in0=igfc[:, 0:2],
                                                in1=igfc[:, 2:4], op=OP.add)
                    nc.scalar.activation(out=tct[:], in_=c32[:], func=AF.Tanh)
                    nc.vector.tensor_tensor(out=seqT[:, :, t], in0=ga[:, 6:8],
                                            in1=tct[:], op=OP.mult)

                    if t % 64 == 63 and 2 <= t // 64 + 2 <= 7:
                        win_dma(t // 64 + 2)
                    if t == 127:
                        pending += pool_q(0, True, False)
                    elif t == 255:
                        pending += pool_q(1, False, False)
                    elif t == 383:
                        pending += pool_q(2, False, True)
                        if n_early > 0:
                            pending += tgt_out()
                            pending += mention_mlp(0, i0)
                            for (n, c0, nb, ni) in BLKS[:n_early]:
                                pending += prod_blk(n, c0, nb, ni)
                                pending += h1_blk(n, c0, nb, ni)
                                pending += h2_blk(n, c0, nb, ni)
                    if t >= 129:
                        drain(2 if t < 385 else 4)
                drain(len(pending))

                # ---- late pass: q3 pool, final tgt, rest of E/F --------------
                for th in pool_q(3, False, True):
                    th()
                for th in tgt_out():
                    th()
                for th in mention_mlp(i0, M) if i0 < M else []:
                    th()
                # ms head + msi/msj
                pms = hps.tile([1, 500], f32, tag="pps", name="pms")
                for k in range(4):
                    nc.tensor.matmul(out=pms[:, 0:M], lhsT=wmv_sb[:, k:k + 1],
                                     rhs=m2T[:, k, :],
                                     start=(k == 0), stop=(k == 3))
                nc.vector.tensor_copy(out=ms_sb[:], in_=pms[:, 0:M])
                nc.sync.dma_start(out=ms_d[:], in_=ms_sb[:])
                nc.sync.dma_start(out=msi_sb[:], in_=ms_d[:])
                nc.sync.dma_start(
                    out=msj_sb[P:M, :],
                    in_=bass.AP(tensor=ms_d.tensor, offset=0,
                                ap=[[1, M - P], [1, P]]))
                nc.sync.dma_start(
                    out=msj_sb[0:P, :],
                    in_=bass.AP(tensor=ms_d.tensor, offset=0,
                                ap=[[0, P], [1, P]]))
                for (n, c0, nb, ni) in BLKS[n_early:]:
                    for th in prod_blk(n, c0, nb, ni):
                        th()
                    for th in h1_blk(n, c0, nb, ni):
                        th()
                    for th in h2_blk(n, c0, nb, ni):
                        th()
                nc.sync.dma_start(
                    out=psM_sb[:],
                    in_=bass.AP(tensor=ps_d.tensor, offset=0,
                                ap=[[P, M], [1, P]]))

                # ---- phase G: scores + softmax -------------------------------
                sc = singles.tile([128, P + 1], f32)
                nc.vector.tensor_tensor(out=sc[:, 0:P], in0=psM_sb[:],
                                        in1=msj_sb[:], op=OP.add)
                nc.vector.tensor_tensor(out=sc[:, 0:P], in0=sc[:, 0:P],
                                        in1=mask_sb[:], op=OP.add)
                nc.vector.tensor_scalar_mul(sc[:, P:P + 1], msi_sb[:], -1.0)
                mx = singles.tile([128, 1], f32)
                nc.vector.tensor_reduce(out=mx[:], in_=sc[:],
                                        axis=mybir.AxisListType.X,
                                        op=OP.max, negate=True)
                ex = singles.tile([128, P + 1], f32)
                sm = singles.tile([128, 1], f32)
                nc.scalar.activation(out=ex[:], in_=sc[:], func=AF.Exp,
                                     bias=mx[:], accum_out=sm[:])
                rs = singles.tile([128, 1], f32)
                nc.vector.reciprocal(out=rs[:], in_=sm[:])
                ot = singles.tile([128, P + 1], f32)
                nc.vector.tensor_scalar_mul(ot[:], ex[:], rs[:])
                nc.sync.dma_start(out=out_d[:], in_=ot[:])

    nc.compile()
    return nc


# -------------------------------------------------------------------- entry --
def kernel(**inputs):
    import os
    from concourse.bass_utils import run_bass_kernel_spmd

    starts = np.asarray(inputs["span_starts"], np.int64)
    lens = np.asarray(inputs["span_lengths"], np.int64)
    ends_max = np.clip(starts + lens, 0, W).max(axis=0)  # max over cores
    pmax = np.maximum.accumulate(ends_max)
    i0 = 0
    for i in range(10, M + 1, 10):
        if pmax[i - 1] <= 384:
            i0 = i
    if i0 < 20:
        i0 = 0

    if ("nc", i0) not in _CACHE:
        _CACHE[("nc", i0)] = _build_program(i0)
    nc = _CACHE[("nc", i0)]

    shared = _prep_shared(inputs)
    in_maps = []
    for b in range(NCORES):
        m = dict(shared)
        m.update(_prep_core(inputs, b))
        in_maps.append(m)

    trace = bool(os.environ.get("COREF_TRACE"))
    res = run_bass_kernel_spmd(nc, in_maps, core_ids=list(range(NCORES)),
                               trace=trace)
    kernel.last_exec_ns = res.exec_time_ns
    kernel.last_results = res
    out = np.stack([res.results[i]["o"] for i in range(NCORES)])
    return out.astype(np.float32)


if __name__ == "__main__":
    import jax
    jax.config.update("jax_platforms", "cpu")
    import reference as ref
    inputs = ref.setup_inputs()
    expected = np.asarray(jax.device_get(ref.reference(**inputs)))
    got = kernel(**{k: np.asarray(v) for k, v in inputs.items()})
    err = np.abs(got - expected)
    print("max_abs_err:", err.max(), " rel@scale:", err.max() / np.abs(expected).max())
